# revision 51
# baseline (speedup 1.0000x reference)
"""Trainium2 Bass kernel for nn_MultiHeadAttention_45612552683890.

Math: the reference computes
    q = x*W_q; k = x*W_k; v = x*W_v            (broadcast elementwise)
    scores = (q @ k) / sqrt(E)                 # [B,H,I,I]
    attn   = softmax(scores, axis=2)           # normalizes over axis 2 (rows i)
    emb    = sum_i (attn @ v)                  # [B,H,E], sum over axis 2
    out    = emb @ mlp_w.T + mlp_b

Because softmax normalizes over the SAME axis (i) that is summed afterwards,
sum_i attn[b,h,i,j] == 1 for every (b,h,j).  Therefore
    emb[b,h,e] = sum_j v[b,h,j,e] = sum_j x[b,j,e] * W_v[h,j,e]
exactly (verified ~6e-7 relative error vs the fp32 reference).  Q/K/softmax are
dead computation.  The kernel computes only:
    emb[b,h,e] = sum_j x[b,j,e]*W_v[h,j,e];   out = emb @ mlp_w.T + mlp_b

Sharding: e (embedding axis, 512) is split 8 ways -> 64 columns per core.
Each core computes its rank-64 contribution to the final Linear:
    partial_c[bh, f] = sum_{e in shard_c} emb[bh, e] * mlp_w[f, e]
and the host sums the 8 partials and adds the bias (the e-contraction of the
Linear is distributed across cores; summing partials is the unshard step).

Per-core device program (everything fp32):
  stage 1: for jt in 0..3 (j tiles of 128), for e' in 0..63:
           matmul(K=128 j, M=16 h (stationary W slice), N=16 b (moving x
           slice)) accumulating over jt into PSUM (one bank per e'-quarter)
           -> emb[h, e', b]; per-quarter drains overlap later matmuls
  stage 2: PE transposes emb -> embT[e', b, h] (8 per PSUM bank, one DVE
           drain per group of 8)
  stage 3: matmul(K=64 e', M=128 bh) against mlp_w^T in N=256/128 f-chunks
           whose accumulators reuse the freed stage-1/2 PSUM banks; chunk
           copies alternate ACT/DVE and the four stores ship in data-
           readiness order -> partial out [256, 512]

Implementation notes:
 - x and W_v shards are packed host-side into one contiguous tensor per core,
   grouped as [j, (e-quarter, x|w, b, e16)]; j-tiles 0-2 load as two e-half
   DMAs (512 KB, limits serial HWDGE descriptor generation) and the final
   j-tile as four e-quarter DMAs (256 KB, so the last stage-1 dependency is
   small).  The stream is fully packed and the aux columns load last.
 - matmul/LDWEIGHTS only supports one sync-wait in codegen; bacc.Bacc.compile
   (generate_event_semaphores) splits multi-waits, and the DMA layout keeps
   the common case at one wait anyway.
 - start=True clears has_written for the WHOLE PSUM bank, so it appears only
   on the first matmul touching each bank; after that, per-element
   has_written gives overwrite-on-first-touch / accumulate-on-rest.
 - PSUM budget is exactly 8 banks: 4 emb quarters + 2 mlpT-transpose +
   2 embT-transpose slots; stage-3 accumulators reuse them with Tile's WAR
   tracking, keeping PE writes and engine reads on different banks (a
   same-bank overlap is a fatal hardware collision).
 - the 128x128 identity used by PE transposes ships from the host in the aux
   tensor (no gpsimd involvement).
"""

import numpy as np

B, H, J, E, F = 16, 16, 512, 512, 512
NCORES = 8
ES = E // NCORES  # 64 e-columns per core
EH = ES // 2  # 32 e-columns per PSUM bank
EQ = ES // 4  # 16 e-columns per DMA chunk (256 KB chunks)
JT = J // 128  # 4 j-tiles

_CACHED = {}


def _build_module():
    import concourse.bacc as bacc
    import concourse.mybir as mybir
    from concourse.tile import TileContext

    f32 = mybir.dt.float32
    nc = bacc.Bacc("TRN2", target_bir_lowering=False, debug=False)

    XWQ = 2 * B * EQ  # 512 columns per e-quarter chunk
    AUXW = 4 * ES + 128  # 384: packed mlp_w shard + 128x128 identity
    xwa_d = nc.dram_tensor("xwa", (J, 4 * XWQ + AUXW), f32, kind="ExternalInput")
    out_d = nc.dram_tensor("out", (B * H, F), f32, kind="ExternalOutput")

    xwa_ap = xwa_d.ap().rearrange("(jt p) c -> jt p c", p=128)

    with TileContext(nc) as tc:
        with (
            tc.tile_pool(name="load", bufs=1) as lpool,
            tc.tile_pool(name="work", bufs=1) as wpool,
            tc.tile_pool(name="ps_emb", bufs=1, space="PSUM") as ps_emb_pool,
            tc.tile_pool(name="ps_mt", bufs=2, space="PSUM") as ps_mt_pool,
            tc.tile_pool(name="ps_et", bufs=2, space="PSUM") as ps_et_pool,
        ):
            # ---- loads (HWDGE sync ring is FIFO: j-tiles in consumption
            # order).  jt0-2 load as two e-half chunks (512 KB) to keep the
            # serial HWDGE descriptor-generation budget low; the final j-tile
            # loads as four e-quarter chunks (256 KB) so the last dependency
            # of stage 1 is small.  The aux columns (mlp_w + identity) are
            # only needed for the late mlpT/transpose work, so they load
            # last and keep the critical stream short.
            xw = {}  # (jt, q) -> AP [128, (x|w), b, e16]
            for jt in range(JT):
                if jt < JT - 1:
                    for qp in range(2):
                        t = lpool.tile(
                            [128, 2, 2, B, EQ], f32, name=f"xw{jt}_{qp}"
                        )
                        nc.sync.dma_start(
                            out=t[:],
                            in_=xwa_ap[jt][
                                :, qp * 2 * XWQ : (qp + 1) * 2 * XWQ
                            ].rearrange("p (qq s b e) -> p qq s b e", qq=2, s=2, e=EQ),
                        )
                        xw[(jt, qp * 2)] = t[:, 0]
                        xw[(jt, qp * 2 + 1)] = t[:, 1]
                else:
                    for q in range(4):
                        t = lpool.tile([128, 2, B, EQ], f32, name=f"xw{jt}_{q}")
                        nc.sync.dma_start(
                            out=t[:],
                            in_=xwa_ap[jt][:, q * XWQ : (q + 1) * XWQ].rearrange(
                                "p (s b e) -> p s b e", s=2, e=EQ
                            ),
                        )
                        xw[(jt, q)] = t[:]

            aux_sb = lpool.tile([128, AUXW], f32, name="aux_sb")
            nc.sync.dma_start(out=aux_sb[:], in_=xwa_ap[0][:, 4 * XWQ :])
            mlp_sb = aux_sb[:, : 4 * ES].rearrange("p (ft e) -> p ft e", e=ES)
            ident = aux_sb[:, 4 * ES :]

            # ---- stage 1: emb[h, e', b] = sum_j W[h,j,e']*x[b,j,e']
            # one PSUM tile (bank) per e-quarter: each quarter's drain (a DVE
            # read) overlaps the next quarter's matmuls instead of
            # WAR-stalling them, and only the final 0.39us quarter-drain sits
            # on the critical path
            emb_ps = [
                ps_emb_pool.tile([H, EQ, B], f32, name=f"emb_ps{q}", tag=f"emb_ps{q}")
                for q in range(4)
            ]
            emb_sb = wpool.tile([H, 4, EQ, B], f32)  # [h, (e', b)]
            mlpT = wpool.tile([ES, F], f32)
            for jt in range(JT):
                for q in range(4):
                    for e in range(EQ):
                        nc.tensor.matmul(
                            emb_ps[q][:, e, :],
                            lhsT=xw[(jt, q)][:, 1, :, e],  # W [128 j, 16 h]
                            rhs=xw[(jt, q)][:, 0, :, e],  # x [128 j, 16 b]
                            start=(jt == 0 and e == 0),
                            stop=(jt == JT - 1 and e == EQ - 1),
                            skip_group_check=True,
                        )
                    if jt == JT - 1:
                        # quarter q is final: drain to SBUF while the next
                        # quarter's matmuls are still streaming on PE.  The
                        # LAST quarter's drain is on the critical path and is
                        # split by b-half so transpose group 0 (b 0-7) starts
                        # after only half of it.
                        if q < 3:
                            nc.vector.tensor_copy(
                                out=emb_sb[:, q], in_=emb_ps[q][:]
                            )
                        else:
                            for bh in range(2):
                                nc.vector.tensor_copy(
                                    out=emb_sb[:, q, :, bh * 8 : (bh + 1) * 8],
                                    in_=emb_ps[q][:, :, bh * 8 : (bh + 1) * 8],
                                )

            # ---- mlp_w^T: [128,(ft,e')] -> mlpT [e'(64), f(512)] (emitted
            # after stage 1 so the PE reaches the stage-1 matmuls first; the
            # aux DMA is the last load and mlpT is not needed until stage 3)
            for ft in range(4):
                pt = ps_mt_pool.tile([ES, 128], f32, tag="ps_mt")
                nc.tensor.transpose(pt[:], mlp_sb[:, ft, :], ident)
                nc.scalar.copy(mlpT[:, ft * 128 : (ft + 1) * 128], pt[:])

            # ---- stage 2: transpose per b: [h, 64 e'] -> [64 e', h].  Each
            # group of 8 transposes shares one PSUM bank (start=True only on
            # the first clears it; per-element has_written overwrites the
            # rest), drained by one DVE copy per group.
            embT = wpool.tile([ES, B, H], f32)  # columns = b*16+h
            for g in range(2):
                pt = ps_et_pool.tile([ES, 8, H], f32, tag="ps_et")
                for i in range(8):
                    b = g * 8 + i
                    nc.tensor.matmul(
                        pt[:, i, :],
                        lhsT=emb_sb[:, :, :, b].rearrange("h q e -> h (q e)"),
                        rhs=ident[:H, :H],
                        is_transpose=True,
                        start=(i == 0),
                        stop=(i == 7),
                        skip_group_check=True,
                    )
                # DVE for both: the ACT queue is busy with the mlpT copies,
                # and MLP-mh0 only needs group 0's columns
                nc.vector.tensor_copy(
                    out=embT[:, g * 8 : (g + 1) * 8, :], in_=pt[:]
                )

            # ---- stage 3: partial_out[bh, f] = embT.T @ mlpT, split into
            # N=256 f-chunks so the PSUM->SBUF copies and the output DMAs
            # pipeline behind the PE instead of waiting for the full 512-col
            # matmul.  Each chunk accumulator reuses one of the four banks
            # freed by the emb drains (an engine reading a bank the PE is
            # writing is a fatal collision, so consecutive chunks use
            # different banks).
            FH = F // 2
            FQ = F // 4
            ob = wpool.tile([128, 2, 2, FH], f32, name="ob")  # [bh, mh, fh, f]
            for mh in range(2):
                for fh in range(2):
                    q = mh * 2 + fh
                    if q < 3:
                        po = ps_emb_pool.tile(
                            [128, FH], f32, tag=f"emb_ps{q}", name=f"po{q}"
                        )
                        nc.tensor.matmul(
                            po[:],
                            lhsT=embT[:, mh * 8 : (mh + 1) * 8, :],
                            rhs=mlpT[:, fh * FH : (fh + 1) * FH],
                            start=True,
                            stop=True,
                        )
                        if fh == 0:
                            nc.scalar.copy(ob[:, mh, fh, :], po[:])
                        else:
                            nc.vector.tensor_copy(
                                out=ob[:, mh, fh, :], in_=po[:]
                            )
                    else:
                        # the LAST chunk is tail-critical: split it into two
                        # N=128 pieces on the two ps_et banks (free once the
                        # embT copies land) so the final PSUM->SBUF copy
                        # covers only 128 columns
                        for fq in range(2):
                            po = ps_et_pool.tile(
                                [128, FQ], f32, tag="ps_et", name=f"po3_{fq}"
                            )
                            nc.tensor.matmul(
                                po[:],
                                lhsT=embT[:, mh * 8 : (mh + 1) * 8, :],
                                rhs=mlpT[
                                    :, fh * FH + fq * FQ : fh * FH + (fq + 1) * FQ
                                ],
                                start=True,
                                stop=True,
                            )
                            if fq == 0:
                                nc.scalar.copy(
                                    ob[:, mh, fh, fq * FQ : (fq + 1) * FQ], po[:]
                                )
                            else:
                                nc.vector.tensor_copy(
                                    out=ob[:, mh, fh, fq * FQ : (fq + 1) * FQ],
                                    in_=po[:],
                                )
            # ship the four output pieces in data-readiness order (chunk obs
            # complete as ACT mh0fh0, ACT mh1fh0, DVE mh0fh1, then the split
            # mh1fh1 pair) so the serial HWDGE descriptor generation never
            # waits on data, and the tail-critical final store is only the
            # last 128KB piece
            for mh, fh, eng in (
                (0, 0, nc.scalar),
                (1, 0, nc.sync),
                (0, 1, nc.scalar),
                (1, 1, nc.sync),
            ):
                eng.dma_start(
                    out=out_d.ap()[
                        mh * 128 : (mh + 1) * 128, fh * FH : (fh + 1) * FH
                    ],
                    in_=ob[:, mh, fh, :],
                )
    nc.compile()
    return nc


def _get_module():
    if "nc" not in _CACHED:
        _CACHED["nc"] = _build_module()
    return _CACHED["nc"]


def _pack_inputs(x, W_v, mlp_w):
    """Host-side shard + pack so every DMA source is fully contiguous."""
    XWQ = 2 * B * EQ
    AUXW = 4 * ES + 128
    xs = np.asarray(x, dtype=np.float32).reshape(B, J, E)
    wv = np.asarray(W_v, dtype=np.float32).reshape(H, J, E)
    mw = np.asarray(mlp_w, dtype=np.float32)
    ident = np.eye(128, dtype=np.float32)
    in_maps = []
    for c in range(NCORES):
        xwa = np.zeros((J, 4 * XWQ + AUXW), dtype=np.float32)
        xw = xwa[:, : 4 * XWQ].reshape(J, 4, 2, B, EQ)  # [j, q, x|w, b, e16]
        for q in range(4):
            esl = slice(ES * c + EQ * q, ES * c + EQ * (q + 1))
            xw[:, q, 0] = xs[:, :, esl].transpose(1, 0, 2)  # [j, b, e16]
            xw[:, q, 1] = wv[:, :, esl].transpose(1, 0, 2)  # [j, h, e16]
        aux = xwa[:128, 4 * XWQ :]
        esl = slice(ES * c, ES * (c + 1))
        aux[:, : 4 * ES] = (
            mw[:, esl].reshape(4, 128, ES).transpose(1, 0, 2).reshape(128, 4 * ES)
        )
        aux[:, 4 * ES :] = ident
        in_maps.append({"xwa": xwa})
    return in_maps


def run(x, W_v, mlp_w, mlp_b, trace=False, **spmd_kwargs):
    from concourse.bass_utils import run_bass_kernel_spmd

    nc = _get_module()
    in_maps = _pack_inputs(x, W_v, mlp_w)
    res = run_bass_kernel_spmd(
        nc, in_maps, core_ids=list(range(NCORES)), trace=trace, **spmd_kwargs
    )
    partial = np.zeros((B * H, F), dtype=np.float32)
    for r in res.results:
        partial += r["out"]
    out = partial + np.asarray(mlp_b, dtype=np.float32)[None, :]
    return out.reshape(B, H, F), res


def kernel(x, W_q=None, W_k=None, W_v=None, mlp_w=None, mlp_b=None, **_unused):
    # W_q / W_k are mathematically dead (softmax over the summed axis).
    out, _ = run(x, W_v, mlp_w, mlp_b, trace=False)
    return out
